# revision 37
# baseline (speedup 1.0000x reference)
"""Conv2d(128->256, 3x3, pad 1) + bias on 16x128x56x56, SPMD over 8 TRN2 cores.

Data-parallel over batch: each core convolves 2 images. Per core the conv is
an implicit GEMM: contraction over CIN=128 (the SBUF partition dim), with the
9 kernel taps accumulated into PSUM via start/stop matmul groups.

The width dimension is zero-padded HOST-SIDE (x becomes [.., 56, 58] in
DRAM), so each image DMA lands dense, already in compute layout: kw tap
shifts are pure view offsets into the 58-wide rows and no GPSIMD repack is
needed. Height padding is virtual: the kh taps that would read above/below
the image clamp their output rows instead (row-sliced PSUM sub-views stay
contiguous, which the PE ISA requires; column-sliced ones would not). The
start matmul is the full center tap (kh=kw=1), so every PSUM element is
initialized before any row-clamped tap accumulates. This shaves ~1.2% of PE
rows vs a fully padded image.

Matmul dtype is bfloat16 (rel err 2.35e-3, gate 2e-2). Measured per-tap
[128,8,56] group costs on HW (tools/mb4.py): float32r 364ns (~2 cycles per
output column — the CoreSim model's 1.0 cyc/col for fp32r free>=256 is 2x
optimistic on HW), bf16 307ns (187ns matmul + ~120ns LDWEIGHTS, which
legalization inserts per-matmul for 2-byte dtypes, priced 0 in the cost
model, never overlapped and never deduped even for identical weights —
walrus runs --enable-ldw-opt=false). bf16 also halves input DMA traffic.
The 63 matmuls per (img, co-tile) are the instruction-count minimum: 3136
output columns at the 512-column PSUM-bank cap needs >=7 blocks per tap.

Schedule (per-DMA issue is sub-us on the HWDGE queues — the earlier
"~2.7us per DMA" theory was wrong, the kernel is PE-bound — but low DMA
count keeps queues simple and the schedule proven):
- Weights are stored t-major host-side so each per-t load is one dense
  128-descriptor DMA (vs 1152 descriptors for a t-slice of a k-major
  layout). Image 0 loads in four row chunks (first block's rows first) and
  image 1 in two, all on the ACT queue, ordered so every chunk lands ahead
  of the PE reaching it; bias rides the otherwise-idle SP queue.
- Output stores are consolidated: DVE drains accumulate bias-added blocks
  into one [128, 56, 56] staging tile per (img, t) and a single large DMA
  stores it (measured -19us/body on HW vs per-block stores). The final
  quarter is stored in 32/16/8-row pieces so the kernel tail is short.
- Taps are reordered host-side to [center, ...] so start=True initializes
  the whole PSUM tile; compute is t-outer per image.
- Junk warmup matmuls ramp the PE clock and end right as the weights +
  first image chunk land, so the PE never idles (a PE idle gap resets the
  clock p-state to 2x-3.7x row cost for the next ~3us). With bf16 each
  warmup matmul also pays ~120ns LDWEIGHTS, so n_warmup=20 (not 44) covers
  the same DMA-landing window; 44 overshoots by ~3us (measured).

Measured loop_first slopes (hardware loop at 20k/60k iterations, 8 cores;
the slope cancels per-call PJRT overhead): float32r 102.3 us/iter, bf16
85.0 us/iter (1.20x). The kernel is PE-bound at the measured matmul rates,
which is why alternative DMA plans (ld2/wb load spreading, sq3 store
rotation, tsplit partition-sliced tail store, obf bf16 stores — all kept
behind _build flags) timed identically in interleaved A/B runs.

Other attempts that did NOT win (2026-08-10, HW-measured):
- fp8 e4m3 DoubleRow: ~301ns per instruction at free=448 (LDWEIGHTS for
  256 stationary columns, FWL off, dominates; the model's 0.5 cyc/col is
  ~3x optimistic). That is 150ns/tap — 2x bf16 — but only in uncorrected
  all-fp8 form, which fails the error gate (rel 3.5e-2). Residual-corrected
  variants (rel 1.4e-2, tools/mb2.py) need 12 insts/block and lose.
- Repeated identical weights do NOT skip LDWEIGHTS (tools/mb4.py bf9same),
  so tap-outer reordering cannot amortize the weight loads.
- The gpsimd DMA queue is SWDGE (software descriptor generation, ~1us
  first-byte vs ~0.6us on the sync/scalar HWDGE queues).
"""

import dataclasses

import numpy as np

B, CIN, COUT, H, W = 16, 128, 256, 56, 56
KH, KW = 3, 3
N_CORES = 8
IMGS_PER_CORE = B // N_CORES  # 2
CO_TILES = COUT // 128  # 2
ROWS_PER_BLOCK = 8
N_BLOCKS = H // ROWS_PER_BLOCK  # 7
WP = W + 2  # host-side zero-padded width
HP = H + 2  # host-side zero-padded height (pad_h variants)

# Tap order: full center tap first (start=True covers the whole psum tile),
# then the rest. The host pre-permutes the weight taps into this order.
KORDER = (4, 0, 1, 2, 3, 5, 6, 7, 8)

_COMPILED = {}


def _build(mm_dtype_name: str, n_warmup: int = 44, loop_iters: int = 0,
           ablate: frozenset = frozenset(), pad_h: bool = False,
           loop_first: bool = False, xsp: bool = False, x3: bool = False,
           op3: bool = False, sq: bool = False, x6: bool = False,
           obf: bool = False, ld2: bool = False, sq3: bool = False,
           tout: bool = False, wb: bool = False, tsplit: bool = False,
           drp: bool = False):
    # drp: taps (0,0) and (0,2) run as ONE fp8e4 DoubleRow matmul (both
    #   x and w quantized to e4m3, no correction): 301ns vs 2x307ns bf16,
    #   -313ns/block. Whole-kernel rel err 1.64e-2 (CPU-exact, deterministic
    #   seed) vs the 2e-2 gate.
    # obf: store outputs as bf16 (host upcasts to f32 after gather) — halves
    #   store traffic; adds ~1e-3 output rounding, fine vs the 2e-2 gate
    # ld2: low-DMA-count load plan spread across ACT/SP/Pool queues (per-DMA
    #   issue cost on HW is ~3-4.5us, so count + queue-parallelism dominate)
    # sq3: rotate output stores across SP/ACT/Pool queues
    # tout: timing-only — declare out as Internal so calls skip D2H
    # wb: weights+bias as ONE dense DMA (bias bit-packed into the w tensor)
    # tsplit: final store partition-sliced across 3 queues (fewer
    #   descriptors per DMA -> shorter issue latency in the kernel tail)
    # ablate flags (timing-only builds): "nomm" (input DMAs only),
    # "nodrain" (matmuls but no psum drains / output stores)
    # pad_h: host zero-pads H as well; every matmul is then a full
    # [8,56]-free view with no row clamps (+1.2% PE rows)
    # loop_first: timing-only — repeat the FULL first body (warmup +
    # startup loads + compute) in the hardware loop, measuring the
    # graded single-shot path per iteration
    # xsp: issue the first image-0 chunks on the SP queue so their
    # descriptor generation runs in parallel with the weight DMAs' on ACT
    import concourse.bacc as bacc
    import concourse.mybir as mybir
    import concourse.tile as tile

    mm_dt = getattr(mybir.dt, mm_dtype_name)
    f32 = mybir.dt.float32
    u32 = mybir.dt.uint32
    four_byte = mybir.dt.size(mm_dt) == 4
    in_dt = f32 if four_byte else mm_dt
    zero_dt = u32 if four_byte else mybir.dt.uint16
    out_dt = mybir.dt.bfloat16 if obf else f32

    def mm_view(ap):
        return ap.bitcast(mm_dt) if four_byte else ap

    nc = bacc.Bacc("TRN2", target_bir_lowering=False, debug=False,
                   num_devices=N_CORES)
    HH = HP if pad_h else H
    # bias occupies 8 bytes per partition; with wb it rides the w tensor
    bias_elems = 8 // mybir.dt.size(in_dt)
    w_cols = KH * KW * COUT + (bias_elems if wb else 0)
    x_dram = nc.dram_tensor("x", [IMGS_PER_CORE, CIN, HH, WP], in_dt,
                            kind="ExternalInput").ap()
    w_dram = nc.dram_tensor("w", [CIN, w_cols], in_dt,
                            kind="ExternalInput").ap()
    b_dram = None if wb else nc.dram_tensor(
        "b", [128, CO_TILES], f32, kind="ExternalInput").ap()
    f8 = mybir.dt.float8e4
    x8_dram = (nc.dram_tensor("x8", [IMGS_PER_CORE, CIN, HH, WP], f8,
                              kind="ExternalInput").ap() if drp else None)
    w8_dram = (nc.dram_tensor("w8", [CIN, CO_TILES * 2 * 128], f8,
                              kind="ExternalInput").ap() if drp else None)
    out_dram = nc.dram_tensor("out", [IMGS_PER_CORE, COUT, H, W], out_dt,
                              kind="Internal" if tout else
                              "ExternalOutput").ap()
    # tout: a tiny real output so PJRT still blocks on NEFF completion
    dummy_dram = (nc.dram_tensor("tdummy", [128, 2], f32,
                                 kind="ExternalOutput").ap() if tout
                  else None)

    # [ci, (t j o)] -> [ci, t, j, o]; j indexes KORDER (host pre-permuted).
    # t-major so a per-t weight load is one dense 128-descriptor DMA.
    w_dram_v = mm_view(w_dram)[:, :KH * KW * COUT].rearrange(
        "c (t k o) -> c t k o", t=CO_TILES, k=KH * KW)
    assert not wb or ld2, "wb requires the ld2 load plan"

    with tile.TileContext(nc) as tc:
        with (
            tc.tile_pool(name="xp", bufs=2) as x_pool,
            tc.tile_pool(name="wp", bufs=1) as w_pool,
            tc.tile_pool(name="op", bufs=3 if op3 else 2) as out_pool,
            tc.tile_pool(name="ps", bufs=7, space="PSUM") as psum_pool,
            tc.tile_pool(name="wups", bufs=1, space="PSUM") as warm_psum_pool,
        ):
            # PE warmup: junk matmuls on a small memset tile ramp the PE
            # clock while the input/weight DMAs are in flight.
            def emit_warmup():
                junk = w_pool.tile([128, 128], mm_dt, tag="junk")
                nc.gpsimd.memset(junk[:].bitcast(zero_dt), 0)
                wpsum = warm_psum_pool.tile([128, 64], f32)
                for _ in range(n_warmup):
                    nc.tensor.matmul(wpsum[:], junk[:], junk[:, :64],
                                     start=True, stop=True)

            if wb:
                w_all = w_pool.tile([CIN, w_cols], mm_dt)
                w_sb = w_all[:, :KH * KW * COUT].rearrange(
                    "c (t k o) -> c t k o", t=CO_TILES, k=KH * KW)
                b_sb = w_all[:, KH * KW * COUT:].bitcast(f32)
            else:
                w_sb = w_pool.tile([CIN, CO_TILES, KH * KW, 128], mm_dt)
                b_sb = w_pool.tile([128, CO_TILES], f32, tag="bias")
            w8_sb = None
            if drp:
                w8_sb = w_pool.tile([CIN, CO_TILES, 2, 128], f8, tag="w8")

            def load_x(img, chunks, xs, eng=None):
                eng = eng or nc.scalar
                for r0, r1 in chunks:
                    eng.dma_start(xs[:, r0:r1, :],
                                  mm_view(x_dram[img, :, r0:r1, :]))

            def load_w(t):
                nc.scalar.dma_start(w_sb[:, t], w_dram_v[:, t])

            store_ctr = [0]

            def compute(img, x_sb, last=False, x8s=None):
                if "nomm" in ablate:
                    return
                drp_skip = {0, 2} if drp else set()
                for t in range(CO_TILES):
                    # one [128, 56, 56] output staging tile per (img, t);
                    # drains accumulate into it and it is stored with few
                    # large DMAs (descriptor count per store is unchanged at
                    # 128, so the per-store issue cost is amortized 7x).
                    out_sb = out_pool.tile([128, H, W], out_dt)
                    if last and t == CO_TILES - 1:
                        # split the final stores so the tail transfer is small
                        store_at = {3: (0, 32), 5: (32, 48),
                                    N_BLOCKS - 1: (48, H)}
                    else:
                        store_at = {N_BLOCKS - 1: (0, H)}
                    for rb in range(N_BLOCKS):
                        h0 = rb * ROWS_PER_BLOCK
                        psum = psum_pool.tile([128, ROWS_PER_BLOCK, W], f32)
                        for j, k in enumerate(KORDER):
                            if k in drp_skip:
                                continue
                            kh, kw = divmod(k, KW)
                            if pad_h:
                                r0, r1 = 0, ROWS_PER_BLOCK
                                xr0 = h0 + kh
                            else:
                                r0 = 1 if (rb == 0 and kh == 0) else 0
                                r1 = (ROWS_PER_BLOCK - 1
                                      if (rb == N_BLOCKS - 1 and kh == 2)
                                      else ROWS_PER_BLOCK)
                                xr0 = h0 + kh - 1 + r0
                            xr1 = xr0 + (r1 - r0)
                            nc.tensor.matmul(
                                psum[:, r0:r1, :],
                                w_sb[:, t, j, :],
                                x_sb[:, xr0:xr1, kw:kw + W],
                                start=(j == 0),
                                stop=(not drp and j == KH * KW - 1),
                            )
                        if drp:
                            # taps (0,0)+(0,2) as one fp8 DoubleRow matmul:
                            # pair axis = kw offset 0 vs 2 (element stride 2)
                            r0 = 1 if rb == 0 else 0
                            xr0 = h0 - 1 + r0
                            nr = ROWS_PER_BLOCK - r0
                            mov = x8s[:, xr0:xr0 + nr, 0:W].unsqueeze(1) \
                                .broadcast_to([128, 2, nr, W])
                            dims = [list(d) for d in mov.ap]
                            dims[1][0] = 2
                            mov = dataclasses.replace(mov, ap=dims)
                            nc.tensor.matmul(
                                psum[:, r0:ROWS_PER_BLOCK, :],
                                w8_sb[:, t], mov, start=False, stop=True,
                                perf_mode=mybir.MatmulPerfMode.DoubleRow)
                        if "nodrain" in ablate:
                            continue
                        nc.vector.tensor_scalar_add(
                            out_sb[:, h0:h0 + ROWS_PER_BLOCK, :], psum[:],
                            b_sb[:, t:t + 1])
                        if rb in store_at:
                            a, b = store_at[rb]
                            final = (last and t == CO_TILES - 1
                                     and rb == N_BLOCKS - 1)
                            if tsplit and final:
                                # partition-sliced tail store over 3 queues:
                                # fewer descriptors per DMA -> shorter issue
                                # latency on the critical path
                                for qi, (p0, p1) in enumerate(
                                        ((0, 43), (43, 86), (86, 128))):
                                    eng = (nc.sync, nc.scalar,
                                           nc.gpsimd)[qi]
                                    eng.dma_start(
                                        out_dram[img,
                                                 t * 128 + p0:t * 128 + p1,
                                                 a:b, :],
                                        out_sb[p0:p1, a:b, :])
                                continue
                            # sq: alternate store issue between the SP and
                            # (by now idle) ACT queues so issues parallelize
                            # sq3: rotate across SP/ACT/Pool
                            if sq3:
                                eng = (nc.sync, nc.scalar,
                                       nc.gpsimd)[store_ctr[0] % 3]
                            else:
                                eng = (nc.scalar if sq and store_ctr[0] % 2
                                       else nc.sync)
                            store_ctr[0] += 1
                            eng.dma_start(
                                out_dram[img, t * 128:(t + 1) * 128, a:b, :],
                                out_sb[:, a:b, :])

            # Image-0 row chunks: sized so the first block can start ASAP
            # and each later block's rows land ahead of the PE reaching it.
            # Every DMA here is dense (128 descriptors) — descriptor
            # generation is the real per-DMA cost on HW.
            p = 1 if pad_h else 0
            if x6:
                # finest chunking: ~1-block pieces
                q = 2 * p
                x0_chunks = ((0, 9 + q), (9 + q, 17 + q), (17 + q, 25 + q),
                             (25 + q, 33 + q), (33 + q, 44 + q),
                             (44 + q, HH))
            elif x3:
                # fewer, ~2-block chunks: DMA issue (~2.7us each on HW) must
                # keep ahead of the PE burning 1.68us per 8-row block
                x0_chunks = ((0, 17 + 2 * p), (17 + 2 * p, 33 + 2 * p),
                             (33 + 2 * p, HH))
            else:
                x0_chunks = ((0, 9 + 2 * p), (9 + 2 * p, 17 + 2 * p),
                             (17 + 2 * p, 33 + 2 * p), (33 + 2 * p, HH))
            x1_chunks = ((0, 28), (28, HH))

            def body(first):
                x0 = x_pool.tile([CIN, HH, WP], mm_dt, tag="x0")
                x1 = x_pool.tile([CIN, HH, WP], mm_dt, tag="x1")
                if drp:
                    x8_0 = x_pool.tile([CIN, HH, WP], f8, tag="x8_0")
                    x8_1 = x_pool.tile([CIN, HH, WP], f8, tag="x8_1")

                    def drp_loads(with_w8):
                        # all on the (otherwise idle) SP queue so the
                        # proven ACT load order is untouched
                        if with_w8:
                            nc.sync.dma_start(
                                w8_sb[:].rearrange("c t k o -> c (t k o)"),
                                w8_dram)
                        nc.sync.dma_start(x8_0[:], x8_dram[0])
                        nc.sync.dma_start(x8_1[:], x8_dram[1])
                else:
                    x8_0 = x8_1 = None
                if ld2:
                    # minimal-DMA plan: loads spread over 3 queues. w on
                    # ACT (one fused DMA with wb); x0 split so rows 0:19
                    # land first (SP), rest + x1 on Pool; bias trails the
                    # first x0 piece on SP (or rides the w DMA with wb).
                    if first:
                        if wb:
                            nc.scalar.dma_start(w_all[:], mm_view(w_dram))
                            load_x(0, ((0, 19),), x0, eng=nc.sync)
                            load_x(0, ((19, HH),), x0, eng=nc.gpsimd)
                        else:
                            load_w(0)  # ACT
                            load_x(0, ((0, 19),), x0, eng=nc.sync)
                            load_x(0, ((19, HH),), x0, eng=nc.gpsimd)
                            load_w(1)  # ACT
                            nc.sync.dma_start(b_sb[:], b_dram[:])
                        load_x(1, ((0, HH),), x1, eng=nc.gpsimd)
                    else:
                        load_x(0, ((0, 19),), x0, eng=nc.sync)
                        load_x(0, ((19, HH),), x0, eng=nc.gpsimd)
                        load_x(1, ((0, HH),), x1, eng=nc.gpsimd)
                    compute(0, x0)
                    compute(1, x1, last=True)
                    return
                if first:
                    if xsp:
                        load_w(0)
                        load_x(0, x0_chunks[:2], x0, eng=nc.sync)
                        load_w(1)
                        load_x(0, x0_chunks[2:], x0)
                        load_x(1, x1_chunks, x1)
                        nc.sync.dma_start(b_sb[:], b_dram[:])
                    else:
                        load_w(0)
                        nc.sync.dma_start(b_sb[:], b_dram[:])
                        if drp:
                            drp_loads(with_w8=True)
                        load_x(0, x0_chunks, x0)
                        load_w(1)
                        load_x(1, x1_chunks, x1)
                else:
                    load_x(0, x0_chunks, x0)
                    load_x(1, x1_chunks, x1)
                    if drp:
                        drp_loads(with_w8=False)
                compute(0, x0, x8s=x8_0)
                compute(1, x1, last=True, x8s=x8_1)

            def emit_dummy():
                # tout: one tiny real output after the loop so the PJRT
                # call blocks on full NEFF completion
                if dummy_dram is not None:
                    d_sb = w_pool.tile([128, 2], f32, tag="dummy")
                    nc.vector.tensor_copy(d_sb[:], b_sb[:])
                    nc.sync.dma_start(dummy_dram, d_sb[:])

            if loop_iters and loop_first:
                # timing-only: the full single-shot body (warmup + startup
                # loads + compute) repeated in a hardware loop
                with tc.For_i(0, loop_iters, 1):
                    emit_warmup()
                    body(first=True)
                emit_dummy()
            elif loop_iters:
                # timing-only variant: steady-state body in a hardware loop
                if wb:
                    nc.scalar.dma_start(w_all[:], mm_view(w_dram))
                else:
                    load_w(0)
                    load_w(1)
                    nc.sync.dma_start(b_sb[:], b_dram[:])
                with tc.For_i(0, loop_iters, 1):
                    body(first=False)
                emit_dummy()
            else:
                emit_warmup()
                body(first=True)
    nc.compile()
    return nc


PAD_H = False  # host-pad H as well (all matmuls full views, no row clamps)


def _get_nc(mm_dtype_name: str, n_warmup: int = 44, loop_iters: int = 0,
            ablate: frozenset = frozenset(), pad_h: bool | None = None,
            loop_first: bool = False, xsp: bool = False, x3: bool = False,
            op3: bool = False, sq: bool = False, x6: bool = False,
            obf: bool = False, ld2: bool = False, sq3: bool = False,
            tout: bool = False, wb: bool = False, tsplit: bool = False,
            drp: bool = False):
    if pad_h is None:
        pad_h = PAD_H
    key = (mm_dtype_name, n_warmup, loop_iters, ablate, pad_h, loop_first,
           xsp, x3, op3, sq, x6, obf, ld2, sq3, tout, wb, tsplit, drp)
    if key not in _COMPILED:
        _COMPILED[key] = _build(mm_dtype_name, n_warmup=n_warmup,
                                loop_iters=loop_iters, ablate=ablate,
                                pad_h=pad_h, loop_first=loop_first, xsp=xsp,
                                x3=x3, op3=op3, sq=sq, x6=x6, obf=obf,
                                ld2=ld2, sq3=sq3, tout=tout, wb=wb,
                                tsplit=tsplit, drp=drp)
    return _COMPILED[key]


def prep_inputs(x, weight, bias, mm_dtype_name="float32r", wb=False,
                drp=False):
    """Shard/transform full inputs into per-core in_maps."""
    if mm_dtype_name == "bfloat16":
        import ml_dtypes
        in_np = ml_dtypes.bfloat16
    else:
        in_np = np.float32
    x = np.asarray(x, dtype=np.float32).astype(in_np)
    if PAD_H:
        x_pad = np.zeros((B, CIN, HP, WP), dtype=in_np)
        x_pad[:, :, 1:1 + H, 1:1 + W] = x
    else:
        x_pad = np.zeros((B, CIN, H, WP), dtype=in_np)
        x_pad[:, :, :, 1:1 + W] = x
    x = x_pad
    # [co, ci, kh, kw] -> [ci, t, j, co'] with j = KORDER tap permutation,
    # flattened to [ci, 2*9*128] (t-major: per-t loads are dense DMAs)
    w_prep = np.ascontiguousarray(
        np.asarray(weight, dtype=np.float32)
        .reshape(CO_TILES, 128, CIN, KH, KW)
        .transpose(2, 0, 3, 4, 1)
        .reshape(CIN, CO_TILES, KH * KW, 128)[:, :, list(KORDER)]
        .reshape(CIN, KH * KW * COUT).astype(in_np))
    b_prep = np.ascontiguousarray(
        np.asarray(bias, dtype=np.float32).reshape(CO_TILES, 128)
        .transpose(1, 0))
    extra = {}
    if drp:
        import ml_dtypes
        e4 = ml_dtypes.float8_e4m3fn
        # x is already the W-padded (bf16) array here; pad zeros stay zero
        x8 = np.asarray(x, dtype=np.float32).astype(e4)
        # taps (0,0) and (0,2): [ci, t, ktile, co'] t-major, e4m3
        wf = np.asarray(weight, dtype=np.float32)
        w8 = np.stack([wf[:, :, 0, 0], wf[:, :, 0, 2]], axis=-1)  # [co,ci,2]
        w8_prep = np.ascontiguousarray(
            w8.reshape(CO_TILES, 128, CIN, 2)
            .transpose(2, 0, 3, 1)  # [ci, t, k2, co']
            .reshape(CIN, CO_TILES * 2 * 128).astype(e4))
        extra = {"x8": x8, "w8": w8_prep}
    if wb:
        # bit-pack the f32 bias into trailing columns of the w tensor
        b_as_in = b_prep.view(np.uint8).reshape(128, 8).view(in_np)
        w_prep = np.ascontiguousarray(
            np.concatenate([w_prep, b_as_in], axis=1))
        return [
            {"x": x[c * IMGS_PER_CORE:(c + 1) * IMGS_PER_CORE], "w": w_prep}
            for c in range(N_CORES)
        ]
    return [
        {"x": x[c * IMGS_PER_CORE:(c + 1) * IMGS_PER_CORE],
         "w": w_prep, "b": b_prep,
         **({"x8": extra["x8"][c * IMGS_PER_CORE:(c + 1) * IMGS_PER_CORE],
             "w8": extra["w8"]} if drp else {})}
        for c in range(N_CORES)
    ]


def run(x, weight, bias, mm_dtype_name="float32r", trace=False,
        **build_kwargs):
    from concourse.bass_utils import run_bass_kernel_spmd
    nc = _get_nc(mm_dtype_name, **build_kwargs)
    in_maps = prep_inputs(x, weight, bias, mm_dtype_name,
                          wb=build_kwargs.get("wb", False),
                          drp=build_kwargs.get("drp", False))
    res = run_bass_kernel_spmd(nc, in_maps, list(range(N_CORES)), trace=trace)
    out = np.concatenate([res.results[c]["out"] for c in range(N_CORES)],
                         axis=0)
    if out.dtype != np.float32:
        out = out.astype(np.float32)
    return out, res


# Shipped configuration. Measured via the hardware-loop slope method (the
# full single-shot body repeated in a tc.For_i loop at two iteration counts;
# see test.py). LAST_MEASURED_NS is the per-body slope from that method.
# float32r with the prior session's tuned schedule. A/B-interleaved
# hardware-loop runs showed the alternative DMA plans (ld2/sq3/wb/tsplit),
# bf16 inputs, and bf16 stores all time within noise of this config, while
# fp8 DoubleRow is a net loss (LDWEIGHTS for 256 columns dominates), so the
# proven-correct baseline schedule ships.
KERNEL_FLAGS = dict(dt="bfloat16", n_warmup=20)
# loop-slope upper bound for this config (bf16, n_warmup=20): ~82-85us/iter
# (fp32r baseline measured 102.3; bf16 w/ default warmup 85.0; n_warmup=20
# saved a further ~2.6us/iter at the 60k point). Single-shot is lower.
LAST_MEASURED_NS = 85000


def kernel(x, weight, bias):
    flags = dict(KERNEL_FLAGS)
    dt = flags.pop("dt")
    out, _ = run(np.asarray(x), np.asarray(weight), np.asarray(bias),
                 mm_dtype_name=dt, **flags)
    return out

